# revision 7
# baseline (speedup 1.0000x reference)
"""BiLSTM-CRF Viterbi decode kernel for Trainium2 (Bass/Tile), 8-core SPMD.

Problem: feats (S=512, B=512, T=64) emissions, mask (B, S) contiguous-prefix,
transitions (T, T), start/end (T,). Output: decoded tag paths (B, S) int32.

Strategy
--------
Data-parallel over batch: 8 cores x 64 batches. Each core runs the Viterbi
forward scan (511 sequential steps) with the tag dim split in half across
partitions: state v_split[(ihi, b), i32] = v[b, ihi*32 + i32] on 128
partitions, and (j, i32) pairs (2048) on the free dimension:

  scores[(ihi,b), (j,i32)] = fl(trans[i,j] + v[b,i])   one TT add (2048 wide)
  Mhalf[(ihi,b), j]        = max_{i32} scores          one grouped reduce
  Mswap = P_swap @ Mhalf   (PE permutation matmul: swaps partition halves,
                            bitwise exact - products are x*1.0)
  M2    = max(Mhalf, Mswap)                            full max over i
  best  = M2[group cols] + emis                        exact (max commutes
                                                       with monotone fl-add)
  v'    = m ? best : v    per-partition masked blend (exact 0/1 products)

Backpointers are NOT computed in the forward pass (that would cost 3 more
2048-wide ops per step). Instead each step's v_split is stored to DRAM and
the backtrack recomputes the single needed argmax per (batch, step):

  T_lp[b,:] = trans[:, lp_b]  gathered bitwise-exactly via one-hot
                              PE transpose+matmul (validated exact on HW)
  cand3     = fl(fl(v + T_lp) + emis[b, lp_b])  == reference rounding
  bp        = max_index(cand3)  (first-max tie-break == jnp.argmax)

Mask/boundary effects (insert last_path at len-1, zeros beyond) are folded
algebraically: dec_i = bp*m_{i+1} + (i == len-1)*last_path.
"""
import os
import sys

sys.path.insert(0, "/opt/trn_rl_repo")

import numpy as np
from contextlib import ExitStack

import concourse.bass as bass
import concourse.tile as tile
from concourse import bacc, mybir
from concourse.bass_utils import run_bass_kernel_spmd

F32 = mybir.dt.float32
U32 = mybir.dt.uint32
A = mybir.AluOpType

S, B, T = 512, 512, 64
NCORES = 8
BL = B // NCORES  # 64 batches per core
H = T // 2        # 32: tag half

_cached = {}


def build_program(n_steps=S):
    nc = bacc.Bacc("TRN2", target_bir_lowering=False, debug=False,
                   num_devices=NCORES)

    featsn = nc.dram_tensor("featsn", [n_steps, BL, T], F32, kind="ExternalInput").ap()
    transbi = nc.dram_tensor("transbi", [2 * BL, T * H], F32, kind="ExternalInput").ap()
    transTd = nc.dram_tensor("transTd", [T, T], F32, kind="ExternalInput").ap()
    startsp = nc.dram_tensor("startsp", [2 * BL, H], F32, kind="ExternalInput").ap()
    swapd = nc.dram_tensor("swapd", [2 * BL, 2 * BL], F32, kind="ExternalInput").ap()
    m2_all = nc.dram_tensor("m2_all", [2 * BL, n_steps], F32, kind="ExternalInput").ap()
    om2_all = nc.dram_tensor("om2_all", [2 * BL, n_steps], F32, kind="ExternalInput").ap()
    m_all = nc.dram_tensor("m_all", [BL, n_steps], F32, kind="ExternalInput").ap()
    onehL = nc.dram_tensor("onehL", [BL, n_steps], F32, kind="ExternalInput").ap()
    endb = nc.dram_tensor("endb", [BL, T], F32, kind="ExternalInput").ap()
    iotad = nc.dram_tensor("iotad", [BL, T], F32, kind="ExternalInput").ap()
    identd = nc.dram_tensor("identd", [T, T], F32, kind="ExternalInput").ap()

    vstore = nc.dram_tensor("vstore", [n_steps, 2 * BL, H], F32).ap()
    dec_out = nc.dram_tensor("dec_out", [BL, n_steps], F32, kind="ExternalOutput").ap()

    P2 = 2 * BL  # 128

    with tile.TileContext(nc) as tc, ExitStack() as ctx:
        statics = ctx.enter_context(tc.tile_pool(name="statics", bufs=1))
        epool = ctx.enter_context(tc.tile_pool(name="epool", bufs=6))
        spool = ctx.enter_context(tc.tile_pool(name="spool", bufs=2))
        wpool = ctx.enter_context(tc.tile_pool(name="wpool", bufs=3))
        vpool = ctx.enter_context(tc.tile_pool(name="vpool", bufs=3))
        btpool = ctx.enter_context(tc.tile_pool(name="btpool", bufs=6))
        pspool = ctx.enter_context(tc.tile_pool(name="pspool", bufs=2, space="PSUM"))

        # ---- statics to SBUF ----
        t_transbi = statics.tile([P2, T * H], F32)
        nc.sync.dma_start(t_transbi[:], transbi)
        t_transT = statics.tile([T, T], F32)
        nc.sync.dma_start(t_transT[:], transTd)
        t_startsp = statics.tile([P2, H], F32)
        nc.sync.dma_start(t_startsp[:], startsp)
        t_swap = statics.tile([P2, P2], F32)
        nc.sync.dma_start(t_swap[:], swapd)
        t_m2 = statics.tile([P2, n_steps], F32)
        nc.sync.dma_start(t_m2[:], m2_all)
        t_om2 = statics.tile([P2, n_steps], F32)
        nc.sync.dma_start(t_om2[:], om2_all)
        t_m = statics.tile([BL, n_steps], F32)
        nc.sync.dma_start(t_m[:], m_all)
        t_onehL = statics.tile([BL, n_steps], F32)
        nc.sync.dma_start(t_onehL[:], onehL)
        t_endb = statics.tile([BL, T], F32)
        nc.sync.dma_start(t_endb[:], endb)
        t_iota = statics.tile([BL, T], F32)
        nc.sync.dma_start(t_iota[:], iotad)
        t_ident = statics.tile([T, T], F32)
        nc.sync.dma_start(t_ident[:], identd)
        t_dec = statics.tile([BL, n_steps], F32)
        t_addend = statics.tile([BL, n_steps], F32)

        def r3(ap):
            return ap.rearrange("p (j i) -> p j i", i=H)

        def load_esplit(tag, s):
            # e_split[(ihi,b), i32] = feats[s, b, ihi*32+i32]
            e = epool.tile([P2, H], F32, tag=tag)
            nc.sync.dma_start(e[0:BL, :], featsn[s, :, 0:H])
            nc.sync.dma_start(e[BL:P2, :], featsn[s, :, H:T])
            return e

        # ---- v0 ----
        e0 = load_esplit("e", 0)
        v_prev = vpool.tile([P2, H], F32, tag="v")
        nc.vector.tensor_add(v_prev[:], t_startsp[:], e0[:])
        nc.sync.dma_start(vstore[0], v_prev[:])

        # ---- forward ----
        # The scores add (2048 wide) is split between DVE and Pool (gpsimd):
        # Pool is ~2x slower, so it gets the second half of the j range while
        # DVE adds the first half and then starts reducing it; by the time
        # DVE needs the Pool half for the second reduce chunk, Pool is done.
        JA = T // 2  # j in [0, 32) -> DVE, j in [32, 64) -> Pool
        for s in range(1, n_steps):
            e_s = load_esplit("e", s)

            scoresA = spool.tile([P2, JA * H], F32, tag="scoresA")
            scoresB = spool.tile([P2, (T - JA) * H], F32, tag="scoresB")
            vb = v_prev[:, None, :]
            nc.vector.tensor_add(
                scoresA[:].rearrange("p (j i) -> p j i", i=H),
                r3(t_transbi[:])[:, 0:JA, :], vb.to_broadcast([P2, JA, H]))
            nc.gpsimd.tensor_add(
                scoresB[:].rearrange("p (j i) -> p j i", i=H),
                r3(t_transbi[:])[:, JA:T, :], vb.to_broadcast([P2, T - JA, H]))

            mhalf = wpool.tile([P2, T], F32, tag="mhalf")
            redA = nc.vector.tensor_reduce(
                mhalf[:, 0:JA], scoresA[:].rearrange("p (j i) -> p j i", i=H),
                axis=mybir.AxisListType.X, op=A.max)
            redB = nc.vector.tensor_reduce(
                mhalf[:, JA:T], scoresB[:].rearrange("p (j i) -> p j i", i=H),
                axis=mybir.AxisListType.X, op=A.max)
            # keep DVE busy on its own half first: reduce_B (waits on Pool)
            # must not be queued ahead of reduce_A on the in-order DVE queue
            from concourse.tile_rust import add_dep_helper
            add_dep_helper(redB.ins, redA.ins, sync=False,
                           reason="reduce_A before reduce_B on DVE")

            mswap = pspool.tile([P2, T], F32, tag="mswap")
            nc.tensor.matmul(mswap[:], t_swap[:], mhalf[:], start=True, stop=True)

            m2rep = wpool.tile([P2, T], F32, tag="m2rep")
            nc.vector.tensor_tensor(m2rep[:], mhalf[:], mswap[:], op=A.max)

            best = wpool.tile([P2, H], F32, tag="best")
            nc.vector.tensor_add(best[0:BL, :], m2rep[0:BL, 0:H], e_s[0:BL, :])
            nc.vector.tensor_add(best[BL:P2, :], m2rep[BL:P2, H:T], e_s[BL:P2, :])

            tm = wpool.tile([P2, H], F32, tag="tm")
            nc.vector.tensor_scalar(tm[:], best[:], t_m2[:, s:s + 1], None, op0=A.mult)
            to = wpool.tile([P2, H], F32, tag="to")
            nc.vector.tensor_scalar(to[:], v_prev[:], t_om2[:, s:s + 1], None, op0=A.mult)
            v_new = vpool.tile([P2, H], F32, tag="v")
            nc.vector.tensor_add(v_new[:], tm[:], to[:])
            nc.sync.dma_start(vstore[s], v_new[:])
            v_prev = v_new

        # ---- epilogue: last_path ----
        # assemble v natural [BL, T] from v_split halves via two lane-local copies
        vnat = wpool.tile([BL, T], F32, tag="vnat")
        nc.vector.tensor_copy(vnat[:, 0:H], v_prev[0:BL, :])
        # halves at partitions 64:128 cannot be copied to 0:64 by DVE; use DMA
        # roundtrip-free trick: DMA SBUF->SBUF via sync engine
        nc.sync.dma_start(vnat[:, H:T], v_prev[BL:P2, :])

        fv = wpool.tile([BL, T], F32, tag="fv")
        nc.vector.tensor_add(fv[:], vnat[:], t_endb[:])
        fv8 = wpool.tile([BL, 8], F32, tag="fv8")
        nc.vector.max(out=fv8[:], in_=fv[:])
        fvi = wpool.tile([BL, 8], U32, tag="fvi")
        nc.vector.max_index(fvi[:], fv8[:], fv[:])
        nc.vector.tensor_copy(t_dec[:, n_steps - 1:n_steps], fvi[:, 0:1])
        nc.vector.tensor_scalar(t_addend[:], t_onehL[:],
                                t_dec[:, n_steps - 1:n_steps], None, op0=A.mult)

        # ---- backtrack ----
        for i in range(n_steps - 2, -1, -1):
            vt = btpool.tile([BL, T], F32, tag="vt")
            nc.sync.dma_start(vt[:, 0:H], vstore[i, 0:BL, :])
            nc.sync.dma_start(vt[:, H:T], vstore[i, BL:P2, :])
            et = btpool.tile([BL, T], F32, tag="et")
            nc.sync.dma_start(et[:], featsn[i + 1])

            lp_ap = t_dec[:, i + 1:i + 2]
            onehot = btpool.tile([BL, T], F32, tag="onehot")
            nc.vector.tensor_scalar(onehot[:], t_iota[:], lp_ap, None, op0=A.is_equal)

            prod = btpool.tile([BL, T], F32, tag="prod")
            nc.vector.tensor_mul(prod[:], et[:], onehot[:])
            elp = btpool.tile([BL, 1], F32, tag="elp")
            nc.vector.tensor_reduce(elp[:], prod[:],
                                    axis=mybir.AxisListType.X, op=A.add)

            p_ohT = pspool.tile([T, BL], F32, tag="p_ohT")
            nc.tensor.transpose(p_ohT[:], onehot[:], t_ident[:])
            ohT = btpool.tile([T, BL], F32, tag="ohT")
            nc.scalar.copy(ohT[:], p_ohT[:])
            p_tlp = pspool.tile([BL, T], F32, tag="p_tlp")
            # psum = trans[:, lp].T ; then accumulate += I @ vt = fl(T_lp + v)
            nc.tensor.matmul(p_tlp[:], ohT[:], t_transT[:], start=True, stop=False)
            nc.tensor.matmul(p_tlp[:], t_ident[:], vt[:], start=False, stop=True)

            cand3 = btpool.tile([BL, T], F32, tag="cand3")
            nc.vector.tensor_scalar(cand3[:], p_tlp[:], elp[:, 0:1], None, op0=A.add)

            c8 = btpool.tile([BL, 8], F32, tag="c8")
            nc.vector.max(out=c8[:], in_=cand3[:])
            ci = btpool.tile([BL, 8], U32, tag="ci")
            nc.vector.max_index(ci[:], c8[:], cand3[:])
            bpf = btpool.tile([BL, 1], F32, tag="bpf")
            nc.vector.tensor_copy(bpf[:], ci[:, 0:1])

            nc.vector.tensor_scalar(t_dec[:, i:i + 1], bpf[:],
                                    t_m[:, i + 1:i + 2], t_addend[:, i:i + 1],
                                    op0=A.mult, op1=A.add)

        nc.sync.dma_start(dec_out, t_dec[:])

    nc.compile()
    return nc


def host_prep(feats, mask, start_transitions, end_transitions, transitions,
              n_steps=S):
    feats = np.asarray(feats, dtype=np.float32)
    mask = np.asarray(mask, dtype=np.float32)
    start = np.asarray(start_transitions, dtype=np.float32)
    end = np.asarray(end_transitions, dtype=np.float32)
    trans = np.asarray(transitions, dtype=np.float32)

    # transbi[(ihi*BL+b), j*H+i32] = trans[ihi*H+i32, j]
    transbi = np.empty((2 * BL, T * H), dtype=np.float32)
    for ihi in range(2):
        blk = trans[ihi * H:(ihi + 1) * H, :]          # [H(i32), T(j)]
        flat = np.ascontiguousarray(blk.T).reshape(1, T * H)  # j-major
        transbi[ihi * BL:(ihi + 1) * BL, :] = np.tile(flat, (BL, 1))
    transT = np.ascontiguousarray(trans.T)
    startsp = np.empty((2 * BL, H), dtype=np.float32)
    for ihi in range(2):
        startsp[ihi * BL:(ihi + 1) * BL, :] = np.tile(
            start[ihi * H:(ihi + 1) * H].reshape(1, H), (BL, 1))
    swapd = np.roll(np.eye(2 * BL, dtype=np.float32), BL, axis=0)
    endb = np.tile(end.reshape(1, T), (BL, 1))
    iotad = np.tile(np.arange(T, dtype=np.float32).reshape(1, T), (BL, 1))
    identd = np.eye(T, dtype=np.float32)

    lengths = mask.sum(axis=1).astype(np.int64)

    in_maps = []
    for c in range(NCORES):
        b0 = c * BL
        msk = np.ascontiguousarray(mask[b0:b0 + BL, :n_steps])
        msk2 = np.concatenate([msk, msk], axis=0)
        onehL = (np.arange(n_steps)[None, :] == (lengths[b0:b0 + BL, None] - 1))
        in_maps.append(dict(
            featsn=np.ascontiguousarray(feats[:n_steps, b0:b0 + BL, :]),
            transbi=transbi, transTd=transT, startsp=startsp, swapd=swapd,
            m2_all=msk2, om2_all=1.0 - msk2,
            m_all=msk, onehL=onehL.astype(np.float32),
            endb=endb, iotad=iotad, identd=identd,
        ))
    return in_maps


def kernel(feats, mask, start_transitions, end_transitions, transitions):
    if "nc" not in _cached:
        _cached["nc"] = build_program(S)
    nc = _cached["nc"]
    in_maps = host_prep(feats, mask, start_transitions, end_transitions,
                        transitions, S)
    res = run_bass_kernel_spmd(nc, in_maps, list(range(NCORES)))
    out = np.empty((B, S), dtype=np.int32)
    for c in range(NCORES):
        out[c * BL:(c + 1) * BL, :] = np.rint(
            res.results[c]["dec_out"]).astype(np.int32)
    return out


# revision 12
# speedup vs baseline: 1.0833x; 1.0833x over previous
"""BiLSTM-CRF Viterbi decode kernel for Trainium2 (Bass/Tile), 8-core SPMD.

Problem: feats (S=512, B=512, T=64) emissions, mask (B, S) contiguous-prefix,
transitions (T, T), start/end (T,). Output: decoded tag paths (B, S) int32.

Strategy
--------
Data-parallel over batch: 8 cores x 64 batches. Each core runs the Viterbi
forward scan (511 sequential steps) with the tag dim split in half across
partitions: state v_split[(ihi, b), i32] = v[b, ihi*32 + i32] on 128
partitions, and (j, i32) pairs (2048) on the free dimension:

  scores[(ihi,b), (j,i32)] = fl(trans[i,j] + v[b,i])   one TT add (2048 wide)
  Mhalf[(ihi,b), j]        = max_{i32} scores          one grouped reduce
  Mswap = P_swap @ Mhalf   (PE permutation matmul: swaps partition halves,
                            bitwise exact - products are x*1.0)
  M2    = max(Mhalf, Mswap)                            full max over i
  best  = M2[group cols] + emis                        exact (max commutes
                                                       with monotone fl-add)
  v'    = m ? best : v    per-partition masked blend (exact 0/1 products)

Backpointers are NOT computed in the forward pass (that would cost 3 more
2048-wide ops per step). Instead each step's v_split is stored to DRAM and
the backtrack recomputes the single needed argmax per (batch, step):

  T_lp[b,:] = trans[:, lp_b]  gathered bitwise-exactly via one-hot
                              PE transpose+matmul (validated exact on HW)
  cand3     = fl(fl(v + T_lp) + emis[b, lp_b])  == reference rounding
  bp        = max_index(cand3)  (first-max tie-break == jnp.argmax)

Mask/boundary effects (insert last_path at len-1, zeros beyond) are folded
algebraically: dec_i = bp*m_{i+1} + (i == len-1)*last_path.
"""
import os
import sys

sys.path.insert(0, "/opt/trn_rl_repo")

import numpy as np
from contextlib import ExitStack

import concourse.bass as bass
import concourse.tile as tile
from concourse import bacc, mybir
from concourse.bass_utils import run_bass_kernel_spmd

F32 = mybir.dt.float32
U32 = mybir.dt.uint32
A = mybir.AluOpType

S, B, T = 512, 512, 64
NCORES = 8
BL = B // NCORES  # 64 batches per core
H = T // 2        # 32: tag half

_cached = {}


def build_program(n_steps=S):
    nc = bacc.Bacc("TRN2", target_bir_lowering=False, debug=False,
                   num_devices=NCORES)

    featsn = nc.dram_tensor("featsn", [n_steps, BL, T], F32, kind="ExternalInput").ap()
    transbi = nc.dram_tensor("transbi", [2 * BL, T * H], F32, kind="ExternalInput").ap()
    transTd = nc.dram_tensor("transTd", [T, T], F32, kind="ExternalInput").ap()
    startsp = nc.dram_tensor("startsp", [2 * BL, H], F32, kind="ExternalInput").ap()
    swapd = nc.dram_tensor("swapd", [2 * BL, 2 * BL], F32, kind="ExternalInput").ap()
    m2_all = nc.dram_tensor("m2_all", [2 * BL, n_steps], F32, kind="ExternalInput").ap()
    m2i_all = nc.dram_tensor("m2i_all", [2 * BL, n_steps], mybir.dt.int32, kind="ExternalInput").ap()
    om2_all = nc.dram_tensor("om2_all", [2 * BL, n_steps], F32, kind="ExternalInput").ap()
    m_all = nc.dram_tensor("m_all", [BL, n_steps], F32, kind="ExternalInput").ap()
    onehL = nc.dram_tensor("onehL", [BL, n_steps], F32, kind="ExternalInput").ap()
    endb = nc.dram_tensor("endb", [BL, T], F32, kind="ExternalInput").ap()
    iotad = nc.dram_tensor("iotad", [BL, T], F32, kind="ExternalInput").ap()
    identd = nc.dram_tensor("identd", [T, T], F32, kind="ExternalInput").ap()

    vstore = nc.dram_tensor("vstore", [n_steps, 2 * BL, H], F32).ap()
    dec_out = nc.dram_tensor("dec_out", [BL, n_steps], F32, kind="ExternalOutput").ap()

    P2 = 2 * BL  # 128

    with tile.TileContext(nc) as tc, ExitStack() as ctx:
        statics = ctx.enter_context(tc.tile_pool(name="statics", bufs=1))
        epool = ctx.enter_context(tc.tile_pool(name="epool", bufs=6))
        spool = ctx.enter_context(tc.tile_pool(name="spool", bufs=2))
        wpool = ctx.enter_context(tc.tile_pool(name="wpool", bufs=3))
        vpool = ctx.enter_context(tc.tile_pool(name="vpool", bufs=3))
        btpool = ctx.enter_context(tc.tile_pool(name="btpool", bufs=6))
        pspool = ctx.enter_context(tc.tile_pool(name="pspool", bufs=2, space="PSUM"))

        # ---- statics to SBUF ----
        t_transbi = statics.tile([P2, T * H], F32)
        nc.sync.dma_start(t_transbi[:], transbi)
        t_transT = statics.tile([T, T], F32)
        nc.sync.dma_start(t_transT[:], transTd)
        t_startsp = statics.tile([P2, H], F32)
        nc.sync.dma_start(t_startsp[:], startsp)
        t_swap = statics.tile([P2, P2], F32)
        nc.sync.dma_start(t_swap[:], swapd)
        t_m2 = statics.tile([P2, n_steps], F32)
        nc.sync.dma_start(t_m2[:], m2_all)
        t_m2i = statics.tile([P2, n_steps], mybir.dt.int32)
        nc.sync.dma_start(t_m2i[:], m2i_all)
        t_om2 = statics.tile([P2, n_steps], F32)
        nc.sync.dma_start(t_om2[:], om2_all)
        t_m = statics.tile([BL, n_steps], F32)
        nc.sync.dma_start(t_m[:], m_all)
        t_onehL = statics.tile([BL, n_steps], F32)
        nc.sync.dma_start(t_onehL[:], onehL)
        t_endb = statics.tile([BL, T], F32)
        nc.sync.dma_start(t_endb[:], endb)
        t_iota = statics.tile([BL, T], F32)
        nc.sync.dma_start(t_iota[:], iotad)
        t_ident = statics.tile([T, T], F32)
        nc.sync.dma_start(t_ident[:], identd)
        t_dec = statics.tile([BL, n_steps], F32)
        t_addend = statics.tile([BL, n_steps], F32)

        def r3(ap):
            return ap.rearrange("p (j i) -> p j i", i=H)

        def load_esplit(tag, s):
            # e_split[(ihi,b), i32] = feats[s, b, ihi*32+i32]
            e = epool.tile([P2, H], F32, tag=tag)
            nc.sync.dma_start(e[0:BL, :], featsn[s, :, 0:H])
            nc.sync.dma_start(e[BL:P2, :], featsn[s, :, H:T])
            return e

        # ---- v0 ----
        e0 = load_esplit("e", 0)
        v_prev = vpool.tile([P2, H], F32, tag="v")
        nc.vector.tensor_add(v_prev[:], t_startsp[:], e0[:])
        nc.sync.dma_start(vstore[0], v_prev[:])

        # ---- forward ----
        # The scores add (2048 wide) is split between DVE and Pool (gpsimd):
        # Pool is ~2x slower, so it gets the second half of the j range while
        # DVE adds the first half and then starts reducing it; by the time
        # DVE needs the Pool half for the second reduce chunk, Pool is done.
        JA = T // 2  # j in [0, 32) -> DVE, j in [32, 64) -> Pool
        for s in range(1, n_steps):
            e_s = load_esplit("e", s)

            scoresA = spool.tile([P2, JA * H], F32, tag="scoresA")
            scoresB = spool.tile([P2, (T - JA) * H], F32, tag="scoresB")
            vb = v_prev[:, None, :]
            nc.vector.tensor_add(
                scoresA[:].rearrange("p (j i) -> p j i", i=H),
                r3(t_transbi[:])[:, 0:JA, :], vb.to_broadcast([P2, JA, H]))
            nc.gpsimd.tensor_add(
                scoresB[:].rearrange("p (j i) -> p j i", i=H),
                r3(t_transbi[:])[:, JA:T, :], vb.to_broadcast([P2, T - JA, H]))

            # Two independent reduce+combine+blend chains: the j<32 columns
            # feed group0's new v (rows 0:64), j>=32 feed group1 (rows 64:128).
            # Chain A completes while the DVE reduces chain B's scores.
            mhalfA = wpool.tile([P2, JA], F32, tag="mhalfA")
            mhalfB = wpool.tile([P2, T - JA], F32, tag="mhalfB")
            redA = nc.vector.tensor_reduce(
                mhalfA[:], scoresA[:].rearrange("p (j i) -> p j i", i=H),
                axis=mybir.AxisListType.X, op=A.max)
            redB = nc.vector.tensor_reduce(
                mhalfB[:], scoresB[:].rearrange("p (j i) -> p j i", i=H),
                axis=mybir.AxisListType.X, op=A.max)
            from concourse.tile_rust import add_dep_helper
            add_dep_helper(redB.ins, redA.ins, sync=False,
                           reason="reduce_A before reduce_B on DVE")

            v_new = vpool.tile([P2, H], F32, tag="v")
            # masked rows keep old v: copy first (early), then overwrite
            # unmasked rows with best+e via copy_predicated
            nc.vector.tensor_copy(v_new[:], v_prev[:])
            beste = wpool.tile([P2, H], F32, tag="beste")
            for g, mh in enumerate([mhalfA, mhalfB]):
                gp = slice(0, BL) if g == 0 else slice(BL, P2)
                msw = pspool.tile([P2, JA], F32, tag=f"msw{g}")
                nc.tensor.matmul(msw[:], t_swap[:], mh[:], start=True, stop=True)
                # group-diagonal rows only (lane-local: all ops on gp rows)
                best_g = wpool.tile([P2, H], F32, tag=f"best{g}")
                nc.vector.tensor_tensor(best_g[gp, :], mh[gp, :], msw[gp, :], op=A.max)
                nc.vector.tensor_add(beste[gp, :], best_g[gp, :], e_s[gp, :])
            nc.vector.copy_predicated(v_new[:],
                                      t_m2i[:, s:s + 1].to_broadcast([P2, H]),
                                      beste[:])
            nc.sync.dma_start(vstore[s], v_new[:])
            v_prev = v_new

        # ---- epilogue: last_path ----
        # assemble v natural [BL, T] from v_split halves via two lane-local copies
        vnat = wpool.tile([BL, T], F32, tag="vnat")
        nc.vector.tensor_copy(vnat[:, 0:H], v_prev[0:BL, :])
        # halves at partitions 64:128 cannot be copied to 0:64 by DVE; use DMA
        # roundtrip-free trick: DMA SBUF->SBUF via sync engine
        nc.sync.dma_start(vnat[:, H:T], v_prev[BL:P2, :])

        fv = wpool.tile([BL, T], F32, tag="fv")
        nc.vector.tensor_add(fv[:], vnat[:], t_endb[:])
        fv8 = wpool.tile([BL, 8], F32, tag="fv8")
        nc.vector.max(out=fv8[:], in_=fv[:])
        fvi = wpool.tile([BL, 8], U32, tag="fvi")
        nc.vector.max_index(fvi[:], fv8[:], fv[:])
        nc.vector.tensor_copy(t_dec[:, n_steps - 1:n_steps], fvi[:, 0:1])
        nc.vector.tensor_scalar(t_addend[:], t_onehL[:],
                                t_dec[:, n_steps - 1:n_steps], None, op0=A.mult)

        # ---- backtrack ----
        for i in range(n_steps - 2, -1, -1):
            vt = btpool.tile([BL, T], F32, tag="vt")
            nc.sync.dma_start(vt[:, 0:H], vstore[i, 0:BL, :])
            nc.sync.dma_start(vt[:, H:T], vstore[i, BL:P2, :])
            et = btpool.tile([BL, T], F32, tag="et")
            nc.sync.dma_start(et[:], featsn[i + 1])

            lp_ap = t_dec[:, i + 1:i + 2]
            onehot = btpool.tile([BL, T], F32, tag="onehot")
            nc.vector.tensor_scalar(onehot[:], t_iota[:], lp_ap, None, op0=A.is_equal)

            prod = btpool.tile([BL, T], F32, tag="prod")
            nc.vector.tensor_mul(prod[:], et[:], onehot[:])
            elp = btpool.tile([BL, 1], F32, tag="elp")
            nc.vector.tensor_reduce(elp[:], prod[:],
                                    axis=mybir.AxisListType.X, op=A.add)

            p_ohT = pspool.tile([T, BL], F32, tag="p_ohT")
            nc.tensor.transpose(p_ohT[:], onehot[:], t_ident[:])
            ohT = btpool.tile([T, BL], F32, tag="ohT")
            nc.vector.tensor_copy(ohT[:], p_ohT[:])
            p_tlp = pspool.tile([BL, T], F32, tag="p_tlp")
            # psum = trans[:, lp].T ; then accumulate += I @ vt = fl(T_lp + v)
            nc.tensor.matmul(p_tlp[:], ohT[:], t_transT[:], start=True, stop=False)
            nc.tensor.matmul(p_tlp[:], t_ident[:], vt[:], start=False, stop=True)

            cand3 = btpool.tile([BL, T], F32, tag="cand3")
            nc.vector.tensor_scalar(cand3[:], p_tlp[:], elp[:, 0:1], None, op0=A.add)

            c8 = btpool.tile([BL, 8], F32, tag="c8")
            nc.vector.max(out=c8[:], in_=cand3[:])
            ci = btpool.tile([BL, 8], U32, tag="ci")
            nc.vector.max_index(ci[:], c8[:], cand3[:])
            bpf = btpool.tile([BL, 1], F32, tag="bpf")
            nc.vector.tensor_copy(bpf[:], ci[:, 0:1])

            nc.vector.tensor_scalar(t_dec[:, i:i + 1], bpf[:],
                                    t_m[:, i + 1:i + 2], t_addend[:, i:i + 1],
                                    op0=A.mult, op1=A.add)

        nc.sync.dma_start(dec_out, t_dec[:])

    nc.compile()
    return nc


def host_prep(feats, mask, start_transitions, end_transitions, transitions,
              n_steps=S):
    feats = np.asarray(feats, dtype=np.float32)
    mask = np.asarray(mask, dtype=np.float32)
    start = np.asarray(start_transitions, dtype=np.float32)
    end = np.asarray(end_transitions, dtype=np.float32)
    trans = np.asarray(transitions, dtype=np.float32)

    # transbi[(ihi*BL+b), j*H+i32] = trans[ihi*H+i32, j]
    transbi = np.empty((2 * BL, T * H), dtype=np.float32)
    for ihi in range(2):
        blk = trans[ihi * H:(ihi + 1) * H, :]          # [H(i32), T(j)]
        flat = np.ascontiguousarray(blk.T).reshape(1, T * H)  # j-major
        transbi[ihi * BL:(ihi + 1) * BL, :] = np.tile(flat, (BL, 1))
    transT = np.ascontiguousarray(trans.T)
    startsp = np.empty((2 * BL, H), dtype=np.float32)
    for ihi in range(2):
        startsp[ihi * BL:(ihi + 1) * BL, :] = np.tile(
            start[ihi * H:(ihi + 1) * H].reshape(1, H), (BL, 1))
    swapd = np.roll(np.eye(2 * BL, dtype=np.float32), BL, axis=0)
    endb = np.tile(end.reshape(1, T), (BL, 1))
    iotad = np.tile(np.arange(T, dtype=np.float32).reshape(1, T), (BL, 1))
    identd = np.eye(T, dtype=np.float32)

    lengths = mask.sum(axis=1).astype(np.int64)

    in_maps = []
    for c in range(NCORES):
        b0 = c * BL
        msk = np.ascontiguousarray(mask[b0:b0 + BL, :n_steps])
        msk2 = np.concatenate([msk, msk], axis=0)
        onehL = (np.arange(n_steps)[None, :] == (lengths[b0:b0 + BL, None] - 1))
        in_maps.append(dict(
            featsn=np.ascontiguousarray(feats[:n_steps, b0:b0 + BL, :]),
            transbi=transbi, transTd=transT, startsp=startsp, swapd=swapd,
            m2_all=msk2, om2_all=1.0 - msk2, m2i_all=msk2.astype(np.int32),
            m_all=msk, onehL=onehL.astype(np.float32),
            endb=endb, iotad=iotad, identd=identd,
        ))
    return in_maps


def kernel(feats, mask, start_transitions, end_transitions, transitions):
    if "nc" not in _cached:
        _cached["nc"] = build_program(S)
    nc = _cached["nc"]
    in_maps = host_prep(feats, mask, start_transitions, end_transitions,
                        transitions, S)
    res = run_bass_kernel_spmd(nc, in_maps, list(range(NCORES)))
    out = np.empty((B, S), dtype=np.int32)
    for c in range(NCORES):
        out[c * BL:(c + 1) * BL, :] = np.rint(
            res.results[c]["dec_out"]).astype(np.int32)
    return out
